# revision 18
# baseline (speedup 1.0000x reference)
"""Trainium2 Bass kernel for nn_Attention_29661044146348.

Diffusion-style attention block: GroupNorm(32) -> 1x1-conv qkv -> single-head
attention over h*w positions (d = C = 512) -> 1x1-conv out -> residual.
Input x is [8, 512, 64, 64]; batch is data-parallel across the 8 NeuronCores
(one batch element per core), no collectives.

Per-core strategy (v2: fp8 DoubleRow):
  - All large matmuls run in fp8(e4m3) with perf_mode=DoubleRow, packing two
    128-row contraction chunks per PE pass (~2x bf16 rate at free-dim 512).
  - The output projection is folded into V: Vt stores (out_w @ w_v') @ x, so
    attention directly produces the projected output; its bias (+ v-bias) is
    exact via ob_eff because softmax rows sum to 1.
  - Scores are computed transposed, S^T[j, i] (keys on partitions), from
    lhsT = K^T chunks and rhs = Q^T; P = exp(scale*S^T - 3) cast straight to
    fp8 (max score over the input distribution is ~6.1, so exp stays ~22,
    far from the e4m3 Inf at 256; the -3 shift cancels in the softmax ratio).
  - fp8 weights are stored x16 and un-scaled in the PSUM-drain activation to
    stay in e4m3's normal range.
  - Softmax denominators accumulate on the (otherwise idle) Pool engine and
    are partition-reduced with a ones-vector matmul; reciprocal happens after
    PE-broadcast so the DVE works 128 lanes wide.
  - GroupNorm is folded into the projection weights (W*A) and biases (W@B).
"""

import jax
import numpy as np
from jax.experimental.shard_map import shard_map
from jax.sharding import Mesh, NamedSharding, PartitionSpec

import bass_rust
import concourse.bass as bass
import concourse.tile as tile
from concourse import bass2jax, mybir
from concourse.masks import make_identity

F32 = mybir.dt.float32
BF16 = mybir.dt.bfloat16
F8 = mybir.dt.float8e4
DR = mybir.MatmulPerfMode.DoubleRow

C = 512          # channels == attention dim
NT = C // 128    # channel tiles (4)
NP = NT // 2     # channel tile pairs (2)
GROUPS = 32
EPS = 1e-5
ATT_SCALE = float(C) ** -0.5
IB = 512         # attention i-block (queries per block)
SHIFT = 3.0      # score shift before exp (softmax-invariant)
WS = 16.0        # fp8 weight pre-scale


def _split_multi_waits(nc):
    """The staged walrus build rejects >1 sync-wait per instruction; hoist
    extra waits onto single-wait NOPs placed immediately before."""
    ctr = 0
    for bb in nc.main_func.blocks:
        insts = bb.instructions
        i = 0
        while i < len(insts):
            ins = insts[i]
            si = ins.sync_info
            if si is not None:
                waits = list(si.on_wait)
                if len(waits) > 1:
                    si.on_wait = waits[-1:]
                    for w in waits[:-1]:
                        nop = mybir.InstNoOp(name=f"waitsplit-{ctr}", ins=[], outs=[])
                        ctr += 1
                        nop.engine = ins.engine
                        nop.sync_info = bass_rust.SyncInfo(on_wait=[w], on_update=[])
                        nc.register_instruction(nop, overwrite=True)
                        insts.insert(i, nop)
                        i += 1
            i += 1
    return ctr


def build_nc(S):
    import os
    S8 = S // 512     # seq chunks of 512
    JT = S // 128     # attention key chunks
    JP = JT // 2      # key chunk pairs
    NIB = S // IB     # attention query blocks
    NIB_EMIT = int(os.environ.get("ATT_BLOCKS", str(NIB)))
    QKV_CHUNKS = int(os.environ.get("QKV_CHUNKS", str(S8)))

    nc = bass.Bass()
    x_ext = nc.declare_dram_parameter("x", [C, S], F32, isOutput=False)
    gnw_ext = nc.declare_dram_parameter("gn_weight", [C], F32, isOutput=False)
    gnb_ext = nc.declare_dram_parameter("gn_bias", [C], F32, isOutput=False)
    qkvw_ext = nc.declare_dram_parameter("qkv_w", [3 * C, C], F32, isOutput=False)
    qkvb_ext = nc.declare_dram_parameter("qkv_b", [3 * C], F32, isOutput=False)
    outw_ext = nc.declare_dram_parameter("out_w", [C, C], F32, isOutput=False)
    outb_ext = nc.declare_dram_parameter("out_b", [C], F32, isOutput=False)
    out_ext = nc.declare_dram_parameter("out", [C, S], F32, isOutput=True)

    ov = out_ext[:].rearrange("(t p) s -> p t s", p=128)

    with tile.TileContext(nc) as tc:
        with (
            tc.tile_pool(name="consts", bufs=1) as consts,
            tc.tile_pool(name="big", bufs=1) as big,
            tc.tile_pool(name="gn_small", bufs=1) as gn_small,
        ):
            # ---------------- on-chip constants (no DMA) ----------------
            ident = consts.tile([128, 128], F32)
            make_identity(nc, ident)
            ones128 = consts.tile([128, 1], F32)
            nc.vector.memset(ones128, 1.0)
            ones1 = consts.tile([1, 128], F32)
            nc.vector.memset(ones1, 1.0)
            ones128b = consts.tile([128, 1], BF16)
            nc.vector.memset(ones128b, 1.0)
            ones1b = consts.tile([1, 128], BF16)
            nc.vector.memset(ones1b, 1.0)
            ind = consts.tile([128, 8], F32)       # ind[p,g] = (p//16 == g)
            nc.vector.memset(ind, 1.0)
            nc.gpsimd.affine_select(
                out=ind, in_=ind, compare_op=mybir.AluOpType.is_ge, fill=0.0,
                base=0, pattern=[[-16, 8]], channel_multiplier=1)
            nc.gpsimd.affine_select(
                out=ind, in_=ind, compare_op=mybir.AluOpType.is_ge, fill=0.0,
                base=15, pattern=[[16, 8]], channel_multiplier=-1)
            indT = consts.tile([8, 128], F32)
            nc.vector.memset(indT, 1.0)
            nc.gpsimd.affine_select(
                out=indT, in_=indT, compare_op=mybir.AluOpType.is_ge, fill=0.0,
                base=0, pattern=[[1, 128]], channel_multiplier=-16)
            nc.gpsimd.affine_select(
                out=indT, in_=indT, compare_op=mybir.AluOpType.is_ge, fill=0.0,
                base=15, pattern=[[-1, 128]], channel_multiplier=16)
            eps8 = consts.tile([8, 1], F32)
            nc.vector.memset(eps8, EPS)
            nshift = consts.tile([128, 1], F32)
            nc.vector.memset(nshift, -SHIFT)
            ones8 = consts.tile([128, 2, 16], F8)
            nc.vector.memset(ones8, 1.0)

            # ---------------- persistent tensors -------------------------
            xf = big.tile([128, NT, S], F32)      # f32 x (residual + stats)
            x8 = big.tile([128, NT, S], F8)       # fp8 x (projection input)
            kT = big.tile([128, NT, S], F8)       # K^T  [c, s]
            qT = big.tile([128, NT, S], F8)       # Q^T  [c, s]
            Vt = big.tile([128, JT, C], F8)       # (out_w @ V)  [s, c_out]
            P_static = big.tile([128, JT, IB], F8)
            wT8 = big.tile([128, NT, 2 * C], F8)  # (qk weights * A * WS)^T
            Vw8 = big.tile([128, NT, C], F8)      # (out_w@w_v * A * WS)^T

            qkvb12 = consts.tile([128, 3 * NT], F32)
            wv = consts.tile([128, NT], F32)
            bv = consts.tile([128, NT], F32)
            obt = consts.tile([128, NT], F32)
            ob_eff = consts.tile([128, NT], F32)
            qkvb_eff = consts.tile([128, 3 * NT], F32)
            gA16 = consts.tile([128, NT], F32)
            stats8 = gn_small.tile([128, 2, NT], F32)   # per-channel mean, E[x^2]
            stAll = gn_small.tile([128, NT, S8, 6], F32)

            # ------- startup: weight loads+transposes (PE) || x stats ----
            with (
                tc.tile_pool(name="wnat", bufs=3) as wnat,
                tc.tile_pool(name="wtb", bufs=1) as wtbp,
                tc.tile_pool(name="pst", bufs=3, space="PSUM") as pst,
            ):
                wTb = wtbp.tile([128, NT, 3 * C], BF16)   # qkv_w^T bf16
                owTb = wtbp.tile([128, NT, C], BF16)      # out_w^T bf16
                wvn = wtbp.tile([128, NT, C], BF16)       # w_v natural bf16
                W2b = wtbp.tile([128, NT, C], BF16)       # (out_w @ w_v)^T bf16
                for t in range(NT):
                    nc.sync.dma_start(out=xf[:, t, :], in_=x_ext[t * 128:(t + 1) * 128, :])
                    # fp8 cast split between ACT and DVE (Pool CAST is slow)
                    if t % 2 == 0:
                        nc.scalar.copy(x8[:, t, :], xf[:, t, :])
                    else:
                        nc.vector.tensor_copy(x8[:, t, :], xf[:, t, :])
                    for s8 in range(S8):
                        nc.vector.bn_stats(out=stAll[:, t, s8, :],
                                           in_=xf[:, t, s8 * 512:(s8 + 1) * 512])
                    if t == 0:
                        # weight DMAs queue right after the first x tile
                        for r in range(3 * C // 128):
                            wn = wnat.tile([128, C], F32)
                            nc.sync.dma_start(out=wn[:], in_=qkvw_ext[r * 128:(r + 1) * 128, :])
                            if r >= 2 * C // 128:
                                nc.scalar.copy(wvn[:, r - 2 * C // 128, :], wn[:])
                            for c4 in range(NT):
                                psT = pst.tile([128, 128], F32)
                                nc.tensor.transpose(psT[:], wn[:, c4 * 128:(c4 + 1) * 128], ident[:])
                                nc.scalar.copy(wTb[:, c4, r * 128:(r + 1) * 128], psT[:])
                        for r in range(C // 128):
                            wn = wnat.tile([128, C], F32)
                            nc.sync.dma_start(out=wn[:], in_=outw_ext[r * 128:(r + 1) * 128, :])
                            for c4 in range(NT):
                                psT = pst.tile([128, 128], F32)
                                nc.tensor.transpose(psT[:], wn[:, c4 * 128:(c4 + 1) * 128], ident[:])
                                nc.scalar.copy(owTb[:, c4, r * 128:(r + 1) * 128], psT[:])
                        nc.sync.dma_start(out=qkvb12[:], in_=qkvb_ext[:].rearrange("(t p) -> p t", p=128))
                        nc.sync.dma_start(out=wv[:], in_=gnw_ext[:].rearrange("(t p) -> p t", p=128))
                        nc.sync.dma_start(out=bv[:], in_=gnb_ext[:].rearrange("(t p) -> p t", p=128))
                        nc.sync.dma_start(out=obt[:], in_=outb_ext[:].rearrange("(t p) -> p t", p=128))

                # fold out_w into the v projection: W2^T = w_v^T @ out_w^T
                # (independent of GroupNorm -> runs during the x stream)
                for c4 in range(NT):
                    psW = pst.tile([128, C], F32, tag="psW", bufs=2)
                    for mm in range(NT):
                        nc.tensor.matmul(psW[:], wvn[:, mm, c4 * 128:(c4 + 1) * 128],
                                         owTb[:, mm, :],
                                         start=(mm == 0), stop=(mm == NT - 1))
                    nc.scalar.copy(W2b[:, c4, :], psW[:])

                with tc.tile_pool(name="stp", bufs=2) as stp:
                    for t in range(NT):
                        mvt = stp.tile([128, 2], F32)
                        nc.vector.bn_aggr(out=mvt[:], in_=stAll[:, t, :, :])
                        nc.vector.tensor_copy(stats8[:, 0, t:t + 1], mvt[:, 0:1])
                        sqt = stp.tile([128, 1], F32)
                        nc.vector.tensor_mul(sqt[:], mvt[:, 0:1], mvt[:, 0:1])
                        nc.vector.tensor_add(stats8[:, 1, t:t + 1], mvt[:, 1:2], sqt[:])

                # ---------------- GroupNorm combine + fold into weights --
                with tc.tile_pool(name="psg", bufs=1, space="PSUM") as psg:
                    psG = psg.tile([8, 2, NT], F32)
                    nc.tensor.matmul(psG[:], ind[:], stats8[:], start=True, stop=True)
                    gsb = gn_small.tile([8, 2, NT], F32)
                    nc.vector.tensor_scalar_mul(gsb[:], psG[:], 1.0 / 16.0)
                    sq8 = gn_small.tile([8, NT], F32)
                    nc.vector.tensor_mul(sq8[:], gsb[:, 0, :], gsb[:, 0, :])
                    varr = gn_small.tile([8, NT], F32)
                    nc.vector.tensor_sub(varr[:], gsb[:, 1, :], sq8[:])
                    sd8 = gn_small.tile([8, NT], F32)
                    nc.scalar.activation(out=sd8[:], in_=varr[:],
                                         func=mybir.ActivationFunctionType.Sqrt,
                                         bias=eps8[:], scale=1.0)
                    nc.vector.reciprocal(gsb[:, 1, :], sd8[:])
                    psBC = psg.tile([128, 2, NT], F32)
                    nc.tensor.matmul(psBC[:], indT[:], gsb[:], start=True, stop=True)
                    chst = gn_small.tile([128, 2, NT], F32)
                    nc.vector.tensor_copy(chst[:], psBC[:])
                    gA = gn_small.tile([128, NT], F32)
                    nc.vector.tensor_mul(gA[:], chst[:, 1, :], wv[:])
                    tmp4 = gn_small.tile([128, NT], F32)
                    nc.vector.tensor_mul(tmp4[:], chst[:, 0, :], gA[:])
                    gB = gn_small.tile([128, NT], F32)
                    nc.vector.tensor_sub(gB[:], bv[:], tmp4[:])
                    nc.vector.tensor_scalar_mul(gA16[:], gA[:], WS)

                    # fp8 q,k weights (x WS, GN-folded) - emitted first: they
                    # gate the entire qkv phase; bias folds only gate drains
                    for t in range(NT):
                        nc.vector.tensor_scalar_mul(wT8[:, t, :], wTb[:, t, 0:2 * C],
                                                    gA16[:, t:t + 1])
                    for c4 in range(NT):
                        nc.vector.tensor_scalar_mul(Vw8[:, c4, :], W2b[:, c4, :],
                                                    gA16[:, c4:c4 + 1])

                    # fold xn = A*x + B into the projections:
                    #   W @ xn = (W * A[c]) @ x + (W @ B)
                    B2 = gn_small.tile([128, NT, 2], F32)
                    nc.vector.memset(B2[:], 0.0)
                    for c4 in range(NT):
                        nc.vector.tensor_copy(B2[:, c4, 0:1], gB[:, c4:c4 + 1])
                    B2r = gn_small.tile([128, NT, 2], BF16)
                    nc.vector.tensor_copy(B2r[:], B2[:])
                    for o12 in range(3 * NT):
                        psE = psg.tile([128, 2], F32, tag="psE")
                        for c4 in range(NT):
                            nc.tensor.matmul(psE[:], wTb[:, c4, o12 * 128:(o12 + 1) * 128],
                                             B2r[:, c4, :],
                                             start=(c4 == 0), stop=(c4 == NT - 1))
                        nc.vector.tensor_add(qkvb_eff[:, o12:o12 + 1], psE[:, 0:1],
                                             qkvb12[:, o12:o12 + 1])
                    qb = qkvb_eff[:, 0:NT]
                    kb = qkvb_eff[:, NT:2 * NT]
                    vb_eff = qkvb_eff[:, 2 * NT:3 * NT]
                    # effective out bias: out_b + out_w @ vb_eff (att rows sum to 1)
                    vb2 = gn_small.tile([128, NT, 2], F32)
                    nc.vector.memset(vb2[:], 0.0)
                    for c4 in range(NT):
                        nc.vector.tensor_copy(vb2[:, c4, 0:1], vb_eff[:, c4:c4 + 1])
                    vbr = gn_small.tile([128, NT, 2], BF16)
                    nc.vector.tensor_copy(vbr[:], vb2[:])
                    for oc in range(NT):
                        psE = psg.tile([128, 2], F32, tag="psE")
                        for c4 in range(NT):
                            nc.tensor.matmul(psE[:], owTb[:, c4, oc * 128:(oc + 1) * 128],
                                             vbr[:, c4, :],
                                             start=(c4 == 0), stop=(c4 == NT - 1))
                        nc.vector.tensor_add(ob_eff[:, oc:oc + 1], psE[:, 0:1], obt[:, oc:oc + 1])

            # ---------------- qkv projection (fp8 DoubleRow) --------------
            with tc.tile_pool(name="psq", bufs=4, space="PSUM") as psq:
                for s8 in range(QKV_CHUNKS):
                    sl = slice(s8 * 512, (s8 + 1) * 512)
                    for o4 in range(NT):   # K^T
                        psK = psq.tile([128, 512], F32, tag="psq")
                        for u in range(NP):
                            nc.tensor.matmul(psK[:],
                                             wT8[:, 2 * u:2 * u + 2, C + o4 * 128:C + (o4 + 1) * 128],
                                             x8[:, 2 * u:2 * u + 2, sl],
                                             perf_mode=DR, start=(u == 0), stop=(u == NP - 1))
                        nc.scalar.activation(out=kT[:, o4, sl], in_=psK[:],
                                             func=mybir.ActivationFunctionType.Identity,
                                             bias=kb[:, o4:o4 + 1], scale=1.0 / WS)
                    for o4 in range(NT):   # Q^T
                        psQ = psq.tile([128, 512], F32, tag="psq")
                        for u in range(NP):
                            nc.tensor.matmul(psQ[:],
                                             wT8[:, 2 * u:2 * u + 2, o4 * 128:(o4 + 1) * 128],
                                             x8[:, 2 * u:2 * u + 2, sl],
                                             perf_mode=DR, start=(u == 0), stop=(u == NP - 1))
                        nc.scalar.activation(out=qT[:, o4, sl], in_=psQ[:],
                                             func=mybir.ActivationFunctionType.Identity,
                                             bias=qb[:, o4:o4 + 1], scale=1.0 / WS)
                    for j4 in range(4):    # (out_w @ V): keys on partitions
                        psV = psq.tile([128, 512], F32, tag="psq")
                        for u in range(NP):
                            nc.tensor.matmul(psV[:],
                                             x8[:, 2 * u:2 * u + 2,
                                                s8 * 512 + j4 * 128:s8 * 512 + (j4 + 1) * 128],
                                             Vw8[:, 2 * u:2 * u + 2, :],
                                             perf_mode=DR, start=(u == 0), stop=(u == NP - 1))
                        nc.vector.tensor_scalar_mul(Vt[:, s8 * 4 + j4, :], psV[:], 1.0 / WS)

            # ---------------- attention (fp8 DoubleRow, pipelined) --------
            with (
                tc.tile_pool(name="rsp", bufs=2) as rsp,
                tc.tile_pool(name="rbcp", bufs=2) as rbcp,
                tc.tile_pool(name="t1p", bufs=3) as t1p,
                tc.tile_pool(name="osbp", bufs=3) as osbp,
                tc.tile_pool(name="psS", bufs=3, space="PSUM") as psSp,
                tc.tile_pool(name="psO", bufs=2, space="PSUM") as psOp,
                tc.tile_pool(name="psM", bufs=2, space="PSUM") as psMp,
            ):
                P = P_static
                LAG = 1
                for n in range(NIB_EMIT):
                    il = slice(n * IB, (n + 1) * IB)
                    psRB = psMp.tile([128, IB], F32, tag="psRB")
                    # --- QK^T scores + exp, denominator row lag-interleaved
                    for jp in range(JP + LAG):
                        if jp < JP:
                            for half in range(2):
                                j = 2 * jp + half
                                psS = psSp.tile([128, IB], F32, tag="psS")
                                for u in range(NP):
                                    nc.tensor.matmul(psS[:],
                                                     kT[:, 2 * u:2 * u + 2, j * 128:(j + 1) * 128],
                                                     qT[:, 2 * u:2 * u + 2, il],
                                                     perf_mode=DR, start=(u == 0), stop=(u == NP - 1))
                                nc.scalar.activation(out=P[:, j, :], in_=psS[:],
                                                     func=mybir.ActivationFunctionType.Exp,
                                                     bias=nshift[:], scale=ATT_SCALE)
                        if jp >= LAG:
                            p = jp - LAG
                            nc.tensor.matmul(psRB[0:1, :], ones8[:, :, 0:1],
                                             P[:, 2 * p:2 * p + 2, :],
                                             perf_mode=DR, start=(p == 0), stop=(p == JP - 1))
                    rs = rsp.tile([1, IB], BF16)
                    nc.vector.tensor_copy(rs[:], psRB[0:1, :])
                    # --- PV c4-major; per-c4 epilogue overlaps the next chain
                    rbc = None
                    for c4 in range(NT):
                        psO = psOp.tile([128, IB], F32, tag="psO", name="psO")
                        for p in range(JP):
                            nc.tensor.matmul(psO[:],
                                             Vt[:, 2 * p:2 * p + 2, c4 * 128:(c4 + 1) * 128],
                                             P[:, 2 * p:2 * p + 2, :],
                                             perf_mode=DR, start=(p == 0), stop=(p == JP - 1))
                        if c4 == 0:
                            # denominator broadcast + reciprocal (hidden under PV)
                            nc.tensor.matmul(psRB[:, :], ones1b[:], rs[:], start=True, stop=True)
                            rbc = rbcp.tile([128, IB], F32)
                            nc.vector.reciprocal(rbc[:], psRB[:, :])
                        t1 = t1p.tile([128, IB], F32)
                        nc.vector.tensor_mul(t1[:], psO[:], rbc[:])
                        osb = osbp.tile([128, IB], F32)
                        nc.vector.scalar_tensor_tensor(
                            out=osb[:], in0=t1[:], scalar=ob_eff[:, c4:c4 + 1],
                            in1=xf[:, c4, il],
                            op0=mybir.AluOpType.add, op1=mybir.AluOpType.add)
                        nc.sync.dma_start(out=ov[:, c4, il], in_=osb[:])

    _split_multi_waits(nc)
    return nc


_RUNNER_CACHE = {}


class _Runner:
    """Builds the Bass graph once, compiles it through PJRT (shard_map over
    the 8 axon NeuronCores), and allows repeated execution for timing."""

    def __init__(self, S):
        self.S = S
        self.nc = build_nc(S)
        bass2jax.install_neuronx_cc_hook()
        nc = self.nc
        partition_name = (
            nc.partition_id_tensor.name if nc.partition_id_tensor else None
        )
        in_names, out_names, out_avals, zero_outs = [], [], [], []
        for alloc in nc.m.functions[0].allocations:
            if not isinstance(alloc, mybir.MemoryLocationSet):
                continue
            name = alloc.memorylocations[0].name
            if alloc.kind == "ExternalInput":
                if name != partition_name:
                    in_names.append(name)
            elif alloc.kind == "ExternalOutput":
                out_names.append(name)
                shape = tuple(alloc.tensor_shape)
                dtype = mybir.dt.np(alloc.dtype)
                out_avals.append(jax.core.ShapedArray(shape, dtype))
                zero_outs.append(np.zeros(shape, dtype))
        self.in_names = list(in_names)
        self.out_names = out_names
        self.out_avals = out_avals
        self.zero_outs = zero_outs
        all_in_names = in_names + out_names
        if partition_name is not None:
            all_in_names = all_in_names + [partition_name]

        def _body(*args):
            operands = list(args)
            if partition_name is not None:
                operands.append(bass2jax.partition_id_tensor())
            outs = bass2jax._bass_exec_p.bind(
                *operands,
                out_avals=tuple(out_avals),
                in_names=tuple(all_in_names),
                out_names=tuple(out_names),
                lowering_input_output_aliases=(),
                sim_require_finite=True,
                sim_require_nnan=True,
                nc=nc,
            )
            return tuple(outs)

        devices = jax.devices()[:8]
        self.mesh = Mesh(np.asarray(devices), ("core",))
        n_in = len(in_names) + len(out_names)
        self._fn = jax.jit(
            shard_map(
                _body, mesh=self.mesh,
                in_specs=(PartitionSpec("core"),) * n_in,
                out_specs=(PartitionSpec("core"),) * len(out_names),
                check_rep=False,
            )
        )

    def prepare(self, in_maps):
        sharding = NamedSharding(self.mesh, PartitionSpec("core"))
        concat = []
        for name in self.in_names:
            concat.append(np.concatenate([np.asarray(m[name]) for m in in_maps], axis=0))
        for z in self.zero_outs:
            concat.append(np.zeros((8 * z.shape[0], *z.shape[1:]), z.dtype))
        return [jax.device_put(a, sharding) for a in concat]

    def run(self, dev_args):
        return self._fn(*dev_args)


def _get_runner(S):
    if S not in _RUNNER_CACHE:
        _RUNNER_CACHE[S] = _Runner(S)
    return _RUNNER_CACHE[S]


def make_in_maps(x, gn_weight, gn_bias, qkv_w, qkv_b, out_w, out_b):
    b, c, h, w = x.shape
    S = h * w
    in_maps = []
    shared = {
        "gn_weight": np.ascontiguousarray(gn_weight, dtype=np.float32),
        "gn_bias": np.ascontiguousarray(gn_bias, dtype=np.float32),
        "qkv_w": np.ascontiguousarray(qkv_w, dtype=np.float32),
        "qkv_b": np.ascontiguousarray(qkv_b, dtype=np.float32),
        "out_w": np.ascontiguousarray(out_w, dtype=np.float32),
        "out_b": np.ascontiguousarray(out_b, dtype=np.float32),
    }
    for i in range(b):
        m = dict(shared)
        m["x"] = np.ascontiguousarray(np.asarray(x)[i].reshape(c, S), dtype=np.float32)
        in_maps.append(m)
    return in_maps


def kernel(x, gn_weight, gn_bias, qkv_w, qkv_b, out_w, out_b):
    x = np.asarray(x)
    b, c, h, w = x.shape
    assert b == 8 and c == C
    S = h * w
    r = _get_runner(S)
    in_maps = make_in_maps(x, gn_weight, gn_bias, qkv_w, qkv_b, out_w, out_b)
    outs = r.run(r.prepare(in_maps))
    idx = r.out_names.index("out")
    arr = np.asarray(outs[idx]).reshape(b, c, h, w)
    return arr.astype(np.float32)
